# revision 41
# baseline (speedup 1.0000x reference)
"""Trainium2 Bass kernel for DigitConvolutionalModel.

Reference computation (B = 32768):
    x: [B, 784] -> reshape [B, 28, 28]
    conv 3x3 valid with w_conv -> [B, 26, 26] -> [B, 676]
    h1 = relu(conv @ W1 + b1)    W1: [676, 100]
    h2 = relu(h1 @ W2 + b2)      W2: [100, 100]
    out = h2 @ W3 + b3           W3: [100, 10]

Strategy
--------
Pure data parallel: batch split 8 ways (4096 rows/core), weights replicated.
The conv is linear, so it is folded into W1 on the host:
    conv(x) @ W1 == x @ (M @ W1) = x @ W1e,  W1e: [784, 100]
removing the conv from the device entirely (exact up to fp rounding).

On-device layout is "transposed": features on SBUF partitions, batch on the
free dimension, so each layer's PSUM output feeds the next matmul directly
as the moving operand.

x ships as float8 e3m4 (1 byte/elem): DMA drops to ~3.2MB/core (~9us),
making the PE the critical path instead of HBM. The matmul runs MIXED
dtype -- fp16 stationary weights x fp8 moving x -- which the PE supports
at bf16 speed (verified on HW: exact vs numpy e3m4 sim, subnormals
included). x is pre-scaled by 2 on the host (folded back via W1e/2) to
push more values out of e3m4's subnormal band; measured end-to-end rel
err 1.5e-2 vs the fp32 reference (threshold 2e-2).

The contraction is chunked 7x112 (784 = 7*112, no 16-row tail), so mm1
is exactly ceil(784/128)=7 accumulating matmuls per unit and the x DMA
layout is one uniform group-major tensor.

Schedule (measured phases on HW):
  * x fully RESIDENT in SBUF (3.2MB): group-major DRAM layout so each
    group is one contiguous ~3.5KB run per partition; all group DMAs
    issued eagerly at t=0 round-robin over the sync+scalar HWDGE rings.
  * PE clock warm-up: the TRN2 HAM clock gate runs the PE at 1.2GHz
    until it has been ~3.4us continuously busy. Dep-free 128-col dummy
    matmuls on a memset tile burn that window during the initial DMA
    wait, so real matmuls start at 2.4GHz.
  * Software pipeline with 1-unit lookahead: per iteration the PE runs
    mm1(u), mm2(u-1), mm3(u-2) while the DVE runs h2(u-2), h1(u-1)
    and the ACT engine runs o(u-2); every input is a full iteration old,
    so the in-order PE queue never stalls and the HAM gate never re-arms.
  * ALL weights+biases ship in ONE fat-descriptor tensor ("wall") that
    lands before the x bulk. Biases ride as f32 bit patterns (bitcast).
  * h1/h2 relu+bias on the DVE; the output bias+copy runs on the
    otherwise-idle ACT engine, emitted one pipeline stage after mm3 with
    a 4-deep output ring so no WAR hazard delays the drain. Outputs are
    fp16 (host upcasts) and ride the idle HWDGE rings.
  * walrus is invoked with --policy=3 (time-aware post-scheduler, via
    the run_command patch below).
Remaining fixed costs: ~2.5us pre-stream queue arming and ~9us of NEFF
epilogue (codegen-inserted per-semaphore clears of all 256 HW semaphores
+ final barriers) that kernel structure cannot remove.
"""

import numpy as np

N_CORES = 8
B = 32768
B_LOC = B // N_CORES          # 4096 rows per core
KC = 7                        # contraction chunks
KP = 112                      # partitions per chunk (7*112 = 784, no tail)
H = 100                       # hidden width
O = 10                        # output width
NT = 512                      # matmul unit width (1 PSUM bank; the ISA
                              # rejects >512-element matmul writes)
XS = 2.0                      # host pre-scale of x (folded into W1e)
# batch columns per x DMA group, with explicit ring assignment
# (S=scalar/ACT ring, Y=sync ring). Small ramping head groups so real
# matmuls start as soon as the queue ramp delivers first bytes (~10us);
# 384s in the middle sized so each ring's serial delivery (~150GB/s)
# stays ahead of the warm PE (204GB/s aggregate demand); tiny last
# groups so the serial drain chain is short. Each dma_start costs
# ~0.7us of its dispatching SEQUENCER (measured DIRECT2D), so all x
# arms are issued upfront and nothing else shares those sequencers
# early: DVE (h1/h2) rides the clean Vector sequencer.
GROUPS = [128, 128, 320, 320] + [384] * 8 + [128]
RINGS = "SSYS" + "YYSYSYSY" + "S"
                              # scalar (~108GB/s, earlier first bytes)
                              # carries the head + ~40% of bytes; sync
                              # (~155GB/s) opens with the wall then the
                              # early-mid groups. Ledger: every group lands
                              # >=0.15us before the dense warm PE needs it
WARMUP_MMS = 28               # 128-col dummies (~107ns cold each): bridge the
                              # PE from ~7.4us to first-data at ~10.6us; ANY
                              # idle gap re-arms the HAM clock gate (measured:
                              # real matmuls drop to 1.2GHz until ~4.2us of
                              # continuous busy), so overshoot is much cheaper
                              # than undershoot. NOTE: walrus re-rolls the
                              # sequencer schedule on every structural change,
                              # shifting DMA arrivals +-1us; this value is
                              # tuned to THIS config's measured schedule
PAD_UNITS = {0: 3, 1: 3, 2: 3, 4: 2, 5: 2}
                              # dep-free 128-col pad matmuls emitted after
                              # mm1(u): absorb DMA-arrival jitter at the
                              # razor-thin ledger points so a slightly late
                              # group can't idle the PE and re-arm the gate

_COMPILED = {}
LAST_RESULTS = None


def _build_nc():
    import concourse.mybir as mybir
    from concourse import bacc
    from concourse.tile import TileContext

    f32 = mybir.dt.float32
    f16 = mybir.dt.float16
    f8 = mybir.dt.float8e3

    nc = bacc.Bacc(
        "TRN2", target_bir_lowering=False, debug=False, num_devices=N_CORES
    )
    # group-major x layout: per partition p the columns are
    # [g][c][j] -> xT[c*KP+p, g0+j], so one group's DMA is a single
    # contiguous (KC*ntd)-byte run per partition (fat descriptors)
    xt = nc.dram_tensor("xt", [KP, KC * B_LOC], f8, kind="ExternalInput")
    # all weights in ONE fat-descriptor tensor so they land before the x
    # bulk: [KP, 906] f16: w1 chunks [c*H+h] | w2 | w3 (zero-padded to
    # [100,100] -- a 10-col stationary makes MATMUL ~100ns slower) |
    # biases (f32 bits)
    WALL = KC * H + (H + H) + 6
    wall = nc.dram_tensor("wall", [KP, WALL], f16, kind="ExternalInput")
    ot = nc.dram_tensor("ot", [O, B_LOC], f16, kind="ExternalOutput")

    add = mybir.AluOpType.add
    amax = mybir.AluOpType.max

    with TileContext(nc) as tc:
        with (
            tc.tile_pool(name="wpool", bufs=1) as wpool,
            tc.tile_pool(name="xpool", bufs=1) as xpool,
            tc.tile_pool(name="epool", bufs=2) as epool,
            tc.tile_pool(name="ppool", bufs=1, space="PSUM") as ppool,
        ):
            # PE warm-up: memset a dummy tile, then dep-free matmuls that
            # keep the PE busy through the HAM activity window while the
            # x stream fills. Results go to a never-read PSUM bank.
            dum = wpool.tile([128, 128], f16, name="dum")
            nc.vector.memset(dum, 0.0)
            pdum = ppool.tile([128, NT], f32, tag="pdum", bufs=1, name="pdum")
            for _ in range(WARMUP_MMS):
                nc.tensor.matmul(
                    pdum[:, 0:128], lhsT=dum[:, 0:128], rhs=dum[:, 0:128],
                    start=True, stop=True,
                )

            # weights open the sync ring, split so mm1(0)'s first five
            # chunks aren't gated by the full wall transfer (SWDGE was
            # tried for this and delays/destabilizes the rings' early
            # window -- the wall data contends with x in the queues)
            wall_t = wpool.tile([KP, WALL], f16, name="wallt")
            nc.sync.dma_start(out=wall_t[:, :500], in_=wall.ap()[:, :500])
            nc.sync.dma_start(out=wall_t[:, 500:], in_=wall.ap()[:, 500:])

            rings = {"S": nc.scalar, "Y": nc.sync}
            xg_t = []
            g0 = 0
            for g, ntd in enumerate(GROUPS):
                ring = rings[RINGS[g]]
                t = xpool.tile([KP, KC * ntd], f8, name=f"xg{g}")
                off = KC * g0
                ring.dma_start(out=t, in_=xt.ap()[:, off : off + KC * ntd])
                xg_t.append(t)
                g0 += ntd

            w2_t = wall_t[:H, KC * H : KC * H + H]
            w3_t = wall_t[:H, KC * H + H : KC * H + H + H]
            bo = KC * H + H + H
            b1_t = wall_t[:H, bo : bo + 2].bitcast(f32)
            b2_t = wall_t[:H, bo + 2 : bo + 4].bitcast(f32)
            b3_t = wall_t[:O, bo + 4 : bo + 6].bitcast(f32)

            # units: (group, batch col, col-in-group, width) -- one
            # PSUM-bank subtile each
            units = []
            g0 = 0
            for g, ntd in enumerate(GROUPS):
                for s0 in range(0, ntd, NT):
                    units.append((g, g0 + s0, s0, min(NT, ntd - s0)))
                g0 += ntd
            N = len(units)

            # software pipeline, 1-unit lookahead: PE queue stays dense
            # (mm2/mm3 of older units run between mm1 blocks, their DVE
            # inputs were produced a full iteration earlier), so the PE
            # never stalls on the DVE chain and the HAM gate stays open.
            def mm1(u):
                g, n0, s0, sw = units[u]
                xg = xg_t[g]
                ntd = GROUPS[g]
                ps1 = ppool.tile([128, NT], f32, tag="ps1", bufs=2,
                                 name=f"ps1_{u}")
                for c in range(KC):
                    nc.tensor.matmul(
                        ps1[:H, :sw],
                        lhsT=wall_t[:, c * H : (c + 1) * H],
                        rhs=xg[:, c * ntd + s0 : c * ntd + s0 + sw],
                        start=(c == 0),
                        stop=(c == KC - 1),
                    )
                return ps1

            # one contiguous SBUF output strip -> a single out-DMA at the
            # end (each dma_start costs ~0.7us of sequencer time plus
            # ~0.26us of NEFF epilogue, so 1 beats 12)
            o_sb = wpool.tile([O, B_LOC], f16, name="o_sb")

            ps1s, ps2s = {}, {}
            h1s, h2s = {}, {}
            for u in range(N + 2):
                # DVE runs h2 (inputs produced a full iteration ago);
                # ACT runs h1 in parallel
                if 0 <= u - 2 < N:
                    _, _, _, sw = units[u - 2]
                    h2 = epool.tile([H, NT], f16, tag="h2", bufs=3,
                                     name=f"h2_{u-2}")
                    nc.vector.tensor_scalar(
                        h2[:, :sw], ps2s[u - 2][:H, :sw], b2_t, 0.0, add, amax
                    )
                    h2s[u - 2] = h2
                if 0 <= u - 1 < N:
                    _, _, _, sw = units[u - 1]
                    h1 = epool.tile([H, NT], f16, tag="h1", bufs=3,
                                     name=f"h1_{u-1}")
                    nc.vector.tensor_scalar(
                        h1[:, :sw], ps1s[u - 1][:H, :sw], b1_t, 0.0,
                        add, amax
                    )
                    h1s[u - 1] = h1
                # PE stream
                if u < N:
                    ps1s[u] = mm1(u)
                    for _ in range(PAD_UNITS.get(u, 0)):
                        nc.tensor.matmul(
                            pdum[:, 0:128], lhsT=dum[:, 0:128],
                            rhs=dum[:, 0:128], start=True, stop=True,
                        )
                if 0 <= u - 1 < N:
                    _, _, _, sw = units[u - 1]
                    ps2 = ppool.tile([128, NT], f32, tag="ps2", bufs=2,
                                     name=f"ps2_{u-1}")
                    nc.tensor.matmul(
                        ps2[:H, :sw], lhsT=w2_t, rhs=h1s[u - 1][:, :sw],
                        start=True, stop=True,
                    )
                    ps2s[u - 1] = ps2
                if 0 <= u - 2 < N:
                    v = u - 2
                    _, n0, _, sw = units[v]
                    ps3 = ppool.tile([128, NT], f32, tag="ps3", bufs=2,
                                     name=f"ps3_{v}")
                    nc.tensor.matmul(
                        ps3[:H, :sw], lhsT=w3_t, rhs=h2s[v][:, :sw],
                        start=True, stop=True,
                    )
                    # output bias+copy on the otherwise-idle ACT engine
                    # (off the mm2/mm3 critical path) into the contiguous
                    # output strip; ship in two pieces so only the last
                    # 256 cols (2 tail units) serialize after the drain
                    nc.scalar.add(
                        o_sb[:, n0 : n0 + sw], ps3[:O, :sw], b3_t
                    )
                    if v == N - 3:
                        cut = n0 + sw
                        nc.sync.dma_start(
                            out=ot.ap()[:, :cut], in_=o_sb[:, :cut]
                        )
                    elif v == N - 1:
                        nc.scalar.dma_start(
                            out=ot.ap()[:, cut:], in_=o_sb[:, cut:]
                        )

    nc.finalize()
    return nc


def _fold_conv_into_w1(w_conv, W1):
    """W1e[784, 100] such that x @ W1e == conv3x3(x) @ W1 (exact linear fold)."""
    W1e = np.zeros((28, 28, H), np.float64)
    W1r = W1.astype(np.float64).reshape(26, 26, H)
    wc = w_conv.astype(np.float64)
    for di in range(3):
        for dj in range(3):
            W1e[di : di + 26, dj : dj + 26, :] += wc[di, dj] * W1r
    return W1e.reshape(784, H).astype(np.float32)


def _patch_walrus_flags():
    """Append extra walrus_driver flags via the in-process run_command."""
    import concourse.bass_utils as BU

    if getattr(BU, "_dcm_patched", False):
        return
    orig = BU.run_command

    def patched(cmd, *a, **kw):
        if cmd and "walrus_driver" in str(cmd[0]) and "codegen" in str(cmd):
            cmd = ["--policy=3" if c == "--policy=0" else c for c in cmd]
        return orig(cmd, *a, **kw)

    BU.run_command = patched
    BU._dcm_patched = True


def kernel(x, w_conv, W1, b1, W2, b2, W3, b3):
    import ml_dtypes
    from concourse.bass_utils import run_bass_kernel_spmd

    _patch_walrus_flags()

    global LAST_RESULTS

    x = np.asarray(x, np.float32)
    W1e = _fold_conv_into_w1(np.asarray(w_conv), np.asarray(W1))
    # packed weights [112, 906] f16: w1 chunks | w2 | w3 (zero-padded to
    # [100,100]) | biases (f32 bits); W1e carries the 1/XS compensation
    # for the x pre-scale
    WALL = KC * H + (H + H) + 6
    wall_dev = np.zeros((KP, WALL), np.float16)
    wall_dev[:, 0 : KC * H] = (
        (W1e / XS).reshape(KC, KP, H).transpose(1, 0, 2)
        .reshape(KP, KC * H).astype(np.float16)
    )
    wall_dev[:H, KC * H : KC * H + H] = np.asarray(W2, np.float32).astype(
        np.float16
    )
    wall_dev[:H, KC * H + H : KC * H + H + O] = np.asarray(
        W3, np.float32
    ).astype(np.float16)
    bias_f32 = np.zeros((KP, 3), np.float32)
    bias_f32[:H, 0] = np.asarray(b1, np.float32)
    bias_f32[:H, 1] = np.asarray(b2, np.float32)
    bias_f32[:O, 2] = np.asarray(b3, np.float32)
    bo = KC * H + H + H
    wall_dev[:, bo : bo + 6] = bias_f32.view(np.float16)

    in_maps = []
    for c in range(N_CORES):
        xs = x[c * B_LOC : (c + 1) * B_LOC]                 # [B_LOC, 784]
        xT = (xs.T * XS).astype(ml_dtypes.float8_e3m4)      # [784, B_LOC] fp8
        # group-major: [112, sum_g KC*ntd_g]; within group g the columns
        # are [c][j] = xT[c*KP + p, g0 + j]
        blocks = []
        g0 = 0
        for ntd in GROUPS:
            blk = (
                xT[:, g0 : g0 + ntd]
                .reshape(KC, KP, ntd)
                .transpose(1, 0, 2)
                .reshape(KP, KC * ntd)
            )
            blocks.append(blk)
            g0 += ntd
        xmain = np.ascontiguousarray(np.concatenate(blocks, axis=1))
        in_maps.append({"xt": xmain, "wall": wall_dev})

    if "nc" not in _COMPILED:
        _COMPILED["nc"] = _build_nc()
    nc = _COMPILED["nc"]

    res = run_bass_kernel_spmd(nc, in_maps, core_ids=list(range(N_CORES)))
    LAST_RESULTS = res

    out = np.empty((B, O), np.float32)
    for c in range(N_CORES):
        out[c * B_LOC : (c + 1) * B_LOC] = res.results[c]["ot"].T.astype(
            np.float32
        )
    return out


# revision 42
# speedup vs baseline: 1.0474x; 1.0474x over previous
"""Trainium2 Bass kernel for DigitConvolutionalModel.

Reference computation (B = 32768):
    x: [B, 784] -> reshape [B, 28, 28]
    conv 3x3 valid with w_conv -> [B, 26, 26] -> [B, 676]
    h1 = relu(conv @ W1 + b1)    W1: [676, 100]
    h2 = relu(h1 @ W2 + b2)      W2: [100, 100]
    out = h2 @ W3 + b3           W3: [100, 10]

Strategy
--------
Pure data parallel: batch split 8 ways (4096 rows/core), weights replicated.
The conv is linear, so it is folded into W1 on the host:
    conv(x) @ W1 == x @ (M @ W1) = x @ W1e,  W1e: [784, 100]
removing the conv from the device entirely (exact up to fp rounding).

On-device layout is "transposed": features on SBUF partitions, batch on the
free dimension, so each layer's PSUM output feeds the next matmul directly
as the moving operand.

x ships as float8 e3m4 (1 byte/elem): DMA drops to ~3.2MB/core (~9us),
making the PE the critical path instead of HBM. The matmul runs MIXED
dtype -- fp16 stationary weights x fp8 moving x -- which the PE supports
at bf16 speed (verified on HW: exact vs numpy e3m4 sim, subnormals
included). x is pre-scaled by 2 on the host (folded back via W1e/2) to
push more values out of e3m4's subnormal band; measured end-to-end rel
err 1.5e-2 vs the fp32 reference (threshold 2e-2).

The contraction is chunked 7x112 (784 = 7*112, no 16-row tail), so mm1
is exactly ceil(784/128)=7 accumulating matmuls per unit and the x DMA
layout is one uniform group-major tensor.

Schedule (measured phases on HW):
  * x fully RESIDENT in SBUF (3.2MB): group-major DRAM layout so each
    group is one contiguous ~3.5KB run per partition; all group DMAs
    issued eagerly at t=0 round-robin over the sync+scalar HWDGE rings.
  * PE clock warm-up: the TRN2 HAM clock gate runs the PE at 1.2GHz
    until it has been ~3.4us continuously busy. Dep-free 128-col dummy
    matmuls on a memset tile burn that window during the initial DMA
    wait, so real matmuls start at 2.4GHz.
  * Software pipeline with 1-unit lookahead: per iteration the PE runs
    mm1(u), mm2(u-1), mm3(u-2) while the DVE runs h2(u-2), h1(u-1)
    and the ACT engine runs o(u-2); every input is a full iteration old,
    so the in-order PE queue never stalls and the HAM gate never re-arms.
  * ALL weights+biases ship in ONE fat-descriptor tensor ("wall") that
    lands before the x bulk. Biases ride as f32 bit patterns (bitcast).
  * h1/h2 relu+bias on the DVE; the output bias+copy runs on the
    otherwise-idle ACT engine, emitted one pipeline stage after mm3 with
    a 4-deep output ring so no WAR hazard delays the drain. Outputs are
    fp16 (host upcasts) and ride the idle HWDGE rings.
  * walrus is invoked with --policy=3 (time-aware post-scheduler, via
    the run_command patch below).
Remaining fixed costs: ~2.5us pre-stream queue arming and ~9us of NEFF
epilogue (codegen-inserted per-semaphore clears of all 256 HW semaphores
+ final barriers) that kernel structure cannot remove.
"""

import numpy as np

N_CORES = 8
B = 32768
B_LOC = B // N_CORES          # 4096 rows per core
KC = 7                        # contraction chunks
KP = 112                      # partitions per chunk (7*112 = 784, no tail)
H = 100                       # hidden width
O = 10                        # output width
NT = 512                      # matmul unit width (1 PSUM bank; the ISA
                              # rejects >512-element matmul writes)
XS = 2.0                      # host pre-scale of x (folded into W1e)
# batch columns per x DMA group, with explicit ring assignment
# (S=scalar/ACT ring, Y=sync ring). Small ramping head groups so real
# matmuls start as soon as the queue ramp delivers first bytes (~10us);
# 384s in the middle sized so each ring's serial delivery (~150GB/s)
# stays ahead of the warm PE (204GB/s aggregate demand); tiny last
# groups so the serial drain chain is short. Each dma_start costs
# ~0.7us of its dispatching SEQUENCER (measured DIRECT2D), so all x
# arms are issued upfront and nothing else shares those sequencers
# early: DVE (h1/h2) rides the clean Vector sequencer.
GROUPS = [128, 128, 320, 320] + [384] * 8 + [128]
RINGS = "SSYS" + "YYSYSYSY" + "S"
                              # scalar (~108GB/s, earlier first bytes)
                              # carries the head + ~40% of bytes; sync
                              # (~155GB/s) opens with the wall then the
                              # early-mid groups. Ledger: every group lands
                              # >=0.15us before the dense warm PE needs it
WARMUP_MMS = 28               # 128-col dummies (~107ns cold each): bridge the
                              # PE from ~7.4us to first-data at ~10.6us; ANY
                              # idle gap re-arms the HAM clock gate (measured:
                              # real matmuls drop to 1.2GHz until ~4.2us of
                              # continuous busy), so overshoot is much cheaper
                              # than undershoot. NOTE: walrus re-rolls the
                              # sequencer schedule on every structural change,
                              # shifting DMA arrivals +-1us; this value is
                              # tuned to THIS config's measured schedule
PAD_UNITS = {2: 3, 4: 2, 5: 2}
                              # dep-free 128-col pad matmuls emitted after
                              # mm1(u): absorb DMA-arrival jitter at the
                              # razor-thin ledger points so a slightly late
                              # group can't idle the PE and re-arm the gate

_COMPILED = {}
LAST_RESULTS = None


def _build_nc():
    import concourse.mybir as mybir
    from concourse import bacc
    from concourse.tile import TileContext

    f32 = mybir.dt.float32
    f16 = mybir.dt.float16
    f8 = mybir.dt.float8e3

    nc = bacc.Bacc(
        "TRN2", target_bir_lowering=False, debug=False, num_devices=N_CORES
    )
    # group-major x layout: per partition p the columns are
    # [g][c][j] -> xT[c*KP+p, g0+j], so one group's DMA is a single
    # contiguous (KC*ntd)-byte run per partition (fat descriptors)
    xt = nc.dram_tensor("xt", [KP, KC * B_LOC], f8, kind="ExternalInput")
    # all weights in ONE fat-descriptor tensor so they land before the x
    # bulk: [KP, 906] f16: w1 chunks [c*H+h] | w2 | w3 (zero-padded to
    # [100,100] -- a 10-col stationary makes MATMUL ~100ns slower) |
    # biases (f32 bits)
    WALL = KC * H + (H + H) + 6
    wall = nc.dram_tensor("wall", [KP, WALL], f16, kind="ExternalInput")
    ot = nc.dram_tensor("ot", [O, B_LOC], f16, kind="ExternalOutput")

    add = mybir.AluOpType.add
    amax = mybir.AluOpType.max

    with TileContext(nc) as tc:
        with (
            tc.tile_pool(name="wpool", bufs=1) as wpool,
            tc.tile_pool(name="xpool", bufs=1) as xpool,
            tc.tile_pool(name="epool", bufs=2) as epool,
            tc.tile_pool(name="ppool", bufs=1, space="PSUM") as ppool,
        ):
            # PE warm-up: memset a dummy tile, then dep-free matmuls that
            # keep the PE busy through the HAM activity window while the
            # x stream fills. Results go to a never-read PSUM bank.
            dum = wpool.tile([128, 128], f16, name="dum")
            nc.vector.memset(dum, 0.0)
            pdum = ppool.tile([128, NT], f32, tag="pdum", bufs=1, name="pdum")
            for _ in range(WARMUP_MMS):
                nc.tensor.matmul(
                    pdum[:, 0:128], lhsT=dum[:, 0:128], rhs=dum[:, 0:128],
                    start=True, stop=True,
                )

            # weights open the sync ring, split so mm1(0)'s first five
            # chunks aren't gated by the full wall transfer (SWDGE was
            # tried for this and delays/destabilizes the rings' early
            # window -- the wall data contends with x in the queues)
            wall_t = wpool.tile([KP, WALL], f16, name="wallt")
            nc.sync.dma_start(out=wall_t[:, :500], in_=wall.ap()[:, :500])
            nc.sync.dma_start(out=wall_t[:, 500:], in_=wall.ap()[:, 500:])

            rings = {"S": nc.scalar, "Y": nc.sync}
            xg_t = []
            g0 = 0
            for g, ntd in enumerate(GROUPS):
                ring = rings[RINGS[g]]
                t = xpool.tile([KP, KC * ntd], f8, name=f"xg{g}")
                off = KC * g0
                ring.dma_start(out=t, in_=xt.ap()[:, off : off + KC * ntd])
                xg_t.append(t)
                g0 += ntd

            w2_t = wall_t[:H, KC * H : KC * H + H]
            w3_t = wall_t[:H, KC * H + H : KC * H + H + H]
            bo = KC * H + H + H
            b1_t = wall_t[:H, bo : bo + 2].bitcast(f32)
            b2_t = wall_t[:H, bo + 2 : bo + 4].bitcast(f32)
            b3_t = wall_t[:O, bo + 4 : bo + 6].bitcast(f32)

            # units: (group, batch col, col-in-group, width) -- one
            # PSUM-bank subtile each
            units = []
            g0 = 0
            for g, ntd in enumerate(GROUPS):
                for s0 in range(0, ntd, NT):
                    units.append((g, g0 + s0, s0, min(NT, ntd - s0)))
                g0 += ntd
            N = len(units)

            # software pipeline, 1-unit lookahead: PE queue stays dense
            # (mm2/mm3 of older units run between mm1 blocks, their DVE
            # inputs were produced a full iteration earlier), so the PE
            # never stalls on the DVE chain and the HAM gate stays open.
            def mm1(u):
                g, n0, s0, sw = units[u]
                xg = xg_t[g]
                ntd = GROUPS[g]
                ps1 = ppool.tile([128, NT], f32, tag="ps1", bufs=2,
                                 name=f"ps1_{u}")
                for c in range(KC):
                    nc.tensor.matmul(
                        ps1[:H, :sw],
                        lhsT=wall_t[:, c * H : (c + 1) * H],
                        rhs=xg[:, c * ntd + s0 : c * ntd + s0 + sw],
                        start=(c == 0),
                        stop=(c == KC - 1),
                    )
                return ps1

            # one contiguous SBUF output strip -> a single out-DMA at the
            # end (each dma_start costs ~0.7us of sequencer time plus
            # ~0.26us of NEFF epilogue, so 1 beats 12)
            o_sb = wpool.tile([O, B_LOC], f16, name="o_sb")

            ps1s, ps2s = {}, {}
            h1s, h2s = {}, {}
            for u in range(N + 2):
                # DVE runs h2 (inputs produced a full iteration ago);
                # ACT runs h1 in parallel
                if 0 <= u - 2 < N:
                    _, _, _, sw = units[u - 2]
                    h2 = epool.tile([H, NT], f16, tag="h2", bufs=3,
                                     name=f"h2_{u-2}")
                    nc.vector.tensor_scalar(
                        h2[:, :sw], ps2s[u - 2][:H, :sw], b2_t, 0.0, add, amax
                    )
                    h2s[u - 2] = h2
                if 0 <= u - 1 < N:
                    _, _, _, sw = units[u - 1]
                    h1 = epool.tile([H, NT], f16, tag="h1", bufs=3,
                                     name=f"h1_{u-1}")
                    nc.vector.tensor_scalar(
                        h1[:, :sw], ps1s[u - 1][:H, :sw], b1_t, 0.0,
                        add, amax
                    )
                    h1s[u - 1] = h1
                # PE stream
                if u < N:
                    ps1s[u] = mm1(u)
                    for _ in range(PAD_UNITS.get(u, 0)):
                        nc.tensor.matmul(
                            pdum[:, 0:128], lhsT=dum[:, 0:128],
                            rhs=dum[:, 0:128], start=True, stop=True,
                        )
                if 0 <= u - 1 < N:
                    _, _, _, sw = units[u - 1]
                    ps2 = ppool.tile([128, NT], f32, tag="ps2", bufs=2,
                                     name=f"ps2_{u-1}")
                    nc.tensor.matmul(
                        ps2[:H, :sw], lhsT=w2_t, rhs=h1s[u - 1][:, :sw],
                        start=True, stop=True,
                    )
                    ps2s[u - 1] = ps2
                if 0 <= u - 2 < N:
                    v = u - 2
                    _, n0, _, sw = units[v]
                    ps3 = ppool.tile([128, NT], f32, tag="ps3", bufs=2,
                                     name=f"ps3_{v}")
                    nc.tensor.matmul(
                        ps3[:H, :sw], lhsT=w3_t, rhs=h2s[v][:, :sw],
                        start=True, stop=True,
                    )
                    # output bias+copy on the otherwise-idle ACT engine
                    # (off the mm2/mm3 critical path) into the contiguous
                    # output strip; ship in two pieces so only the last
                    # 256 cols (2 tail units) serialize after the drain
                    nc.scalar.add(
                        o_sb[:, n0 : n0 + sw], ps3[:O, :sw], b3_t
                    )
                    if v == N - 3:
                        cut = n0 + sw
                        nc.sync.dma_start(
                            out=ot.ap()[:, :cut], in_=o_sb[:, :cut]
                        )
                    elif v == N - 1:
                        nc.scalar.dma_start(
                            out=ot.ap()[:, cut:], in_=o_sb[:, cut:]
                        )

    nc.finalize()
    return nc


def _fold_conv_into_w1(w_conv, W1):
    """W1e[784, 100] such that x @ W1e == conv3x3(x) @ W1 (exact linear fold)."""
    W1e = np.zeros((28, 28, H), np.float64)
    W1r = W1.astype(np.float64).reshape(26, 26, H)
    wc = w_conv.astype(np.float64)
    for di in range(3):
        for dj in range(3):
            W1e[di : di + 26, dj : dj + 26, :] += wc[di, dj] * W1r
    return W1e.reshape(784, H).astype(np.float32)


def _patch_walrus_flags():
    """Append extra walrus_driver flags via the in-process run_command."""
    import concourse.bass_utils as BU

    if getattr(BU, "_dcm_patched", False):
        return
    orig = BU.run_command

    def patched(cmd, *a, **kw):
        if cmd and "walrus_driver" in str(cmd[0]) and "codegen" in str(cmd):
            cmd = ["--policy=3" if c == "--policy=0" else c for c in cmd]
        return orig(cmd, *a, **kw)

    BU.run_command = patched
    BU._dcm_patched = True


def kernel(x, w_conv, W1, b1, W2, b2, W3, b3):
    import ml_dtypes
    from concourse.bass_utils import run_bass_kernel_spmd

    _patch_walrus_flags()

    global LAST_RESULTS

    x = np.asarray(x, np.float32)
    W1e = _fold_conv_into_w1(np.asarray(w_conv), np.asarray(W1))
    # packed weights [112, 906] f16: w1 chunks | w2 | w3 (zero-padded to
    # [100,100]) | biases (f32 bits); W1e carries the 1/XS compensation
    # for the x pre-scale
    WALL = KC * H + (H + H) + 6
    wall_dev = np.zeros((KP, WALL), np.float16)
    wall_dev[:, 0 : KC * H] = (
        (W1e / XS).reshape(KC, KP, H).transpose(1, 0, 2)
        .reshape(KP, KC * H).astype(np.float16)
    )
    wall_dev[:H, KC * H : KC * H + H] = np.asarray(W2, np.float32).astype(
        np.float16
    )
    wall_dev[:H, KC * H + H : KC * H + H + O] = np.asarray(
        W3, np.float32
    ).astype(np.float16)
    bias_f32 = np.zeros((KP, 3), np.float32)
    bias_f32[:H, 0] = np.asarray(b1, np.float32)
    bias_f32[:H, 1] = np.asarray(b2, np.float32)
    bias_f32[:O, 2] = np.asarray(b3, np.float32)
    bo = KC * H + H + H
    wall_dev[:, bo : bo + 6] = bias_f32.view(np.float16)

    in_maps = []
    for c in range(N_CORES):
        xs = x[c * B_LOC : (c + 1) * B_LOC]                 # [B_LOC, 784]
        xT = (xs.T * XS).astype(ml_dtypes.float8_e3m4)      # [784, B_LOC] fp8
        # group-major: [112, sum_g KC*ntd_g]; within group g the columns
        # are [c][j] = xT[c*KP + p, g0 + j]
        blocks = []
        g0 = 0
        for ntd in GROUPS:
            blk = (
                xT[:, g0 : g0 + ntd]
                .reshape(KC, KP, ntd)
                .transpose(1, 0, 2)
                .reshape(KP, KC * ntd)
            )
            blocks.append(blk)
            g0 += ntd
        xmain = np.ascontiguousarray(np.concatenate(blocks, axis=1))
        in_maps.append({"xt": xmain, "wall": wall_dev})

    if "nc" not in _COMPILED:
        _COMPILED["nc"] = _build_nc()
    nc = _COMPILED["nc"]

    res = run_bass_kernel_spmd(nc, in_maps, core_ids=list(range(N_CORES)))
    LAST_RESULTS = res

    out = np.empty((B, O), np.float32)
    for c in range(N_CORES):
        out[c * B_LOC : (c + 1) * B_LOC] = res.results[c]["ot"].T.astype(
            np.float32
        )
    return out
